# revision 31
# baseline (speedup 1.0000x reference)
"""Transformer block (nn_Block_49744311222996) on 8 TRN2 NeuronCores.

Sharding: core c = 2*b + g handles batch b (4 batches) and head-group g
(8 of 16 heads). Attention head-parallel, unnormalized exp + ones-column
denominator. Proj partials ReduceScatter'd (add) over core pairs in TWO
row-chunks so the collective overlaps attention/FFN; each core ends up
owning rows {g*256..g*256+255} U {512+g*256..} of the 1024 query rows.

Perf-critical decisions:
- ln1 affine folded into Wq/Wk/Wv + biases on host; ln2 affine folded
  into lin1 (kernel computes pure normalize z).
- Scores use a zero-padded K=128 stationary (kTz): each head's K^T
  occupies one 64-row half, the other half is zeros, and the moving qT
  carries both heads. This keeps the PE activity monitor seeing
  full-array matmuls -- half-array (K=64) attention otherwise runs the
  whole phase at the throttled 1.2 GHz HAM clock. The zero halves are
  written by the same Act copy that places K^T, via a 0/1 per-partition
  scale mask (no memsets).
- LN1 pipelined with QKV per 128-row tile so the PE never drains.
- Causal-boundary tiles restrict the score/exp/attV column range; only
  the 128x128 diagonal block is mask-multiplied.
- Softmax denominator: Act copies the ones-row to SBUF f32r, PE
  broadcasts via a K=1 f32r matmul, DVE reciprocal_approx_fast (the
  exact InstReciprocal is ~5x slower).
- LN2 runs in two row-chunks interleaved under attention sc=1 / FFN1 so
  its serial stats chain never idles the PE; chunk 0 computes rstd via
  ln+exp (both live in the exp act table -- no table switch mid-phase).
- FFN gelu is a single Act op per f-tile; FFN2 is ft-outer with 8 psum
  accumulators and one full-row lin2 DMA per ft.
- Weight streams ride the GpSimd DMA queue, x/collective/out the Sync
  queue, so startup x tiles and weights transfer in parallel.
"""

import numpy as np
import ml_dtypes

import concourse.mybir as mybir
import concourse.tile as tile
from concourse import bacc
from concourse.bass_utils import run_bass_kernel_spmd

F32 = mybir.dt.float32
F32R = mybir.dt.float32r
BF16 = mybir.dt.bfloat16
AF = mybir.ActivationFunctionType
ALU = mybir.AluOpType

B, T, C = 4, 2048, 1024
H, HS = 16, 64
CUT = 1024
P = 128
NT = T // P       # 16 t-tiles
NCt = C // P      # 8 c-tiles
GH = 8            # heads per core
EW = GH * HS      # 512
EPS = 1e-5
ATT_SCALE = float(C) ** -0.5
NF = 4 * C // P   # 32 f-tiles
SROWS = 512       # rows owned per core after the two reduce-scatters

AF_LN = getattr(AF, "Ln", None) or getattr(AF, "Log", None)


def _ln_stats(nc, pool, xts, width, eps_ap, lnexp):
    """LN stats for a group of [128, width] APs. Returns rstd, nmean.

    lnexp=True computes rstd = exp(-0.5*ln(var+eps)) so the Act table
    set with Exp stays loaded (used mid-attention)."""
    n = len(xts)
    s1 = pool.tile([P, n], F32, tag="s1")
    s2 = pool.tile([P, n], F32, tag="s2")
    for i, xt in enumerate(xts):
        nc.vector.reduce_sum(s1[:, i:i + 1], xt, axis=mybir.AxisListType.X)
        sq = pool.tile([P, width], BF16, tag="sq")
        nc.scalar.activation(sq[:], xt, AF.Square, accum_out=s2[:, i:i + 1])
    mean = pool.tile([P, n], F32, tag="mean")
    nc.vector.tensor_scalar_mul(mean[:], s1[:], 1.0 / width)
    ms = pool.tile([P, n], F32, tag="ms")
    nc.vector.tensor_mul(ms[:], mean[:], mean[:])
    var = pool.tile([P, n], F32, tag="var")
    nc.vector.scalar_tensor_tensor(
        out=var[:], in0=s2[:], scalar=1.0 / width, in1=ms[:],
        op0=ALU.mult, op1=ALU.subtract,
    )
    rstd = pool.tile([P, n], F32, tag="rstd")
    if lnexp:
        lv = pool.tile([P, n], F32, tag="lv")
        nc.scalar.activation(lv[:], var[:], AF_LN, bias=eps_ap)
        nc.scalar.activation(rstd[:], lv[:], AF.Exp, scale=-0.5)
    else:
        sd = pool.tile([P, n], F32, tag="sd")
        nc.scalar.activation(sd[:], var[:], AF.Sqrt, bias=eps_ap)
        nc.vector.reciprocal(rstd[:], sd[:])
    nmean = pool.tile([P, n], F32, tag="nmean")
    nc.vector.scalar_tensor_tensor(
        out=nmean[:], in0=mean[:], scalar=-1.0, in1=rstd[:],
        op0=ALU.mult, op1=ALU.mult,
    )
    return rstd, nmean


def build_nc():
    nc = bacc.Bacc(None, target_bir_lowering=False)

    xb = nc.declare_dram_parameter("xb", [T, C], BF16, isOutput=False)
    xsl = nc.declare_dram_parameter("xslice", [SROWS, C], F32, isOutput=False)
    wq = nc.declare_dram_parameter("wq", [C, EW], BF16, isOutput=False)
    wk = nc.declare_dram_parameter("wk", [C, EW], BF16, isOutput=False)
    wv = nc.declare_dram_parameter("wv", [C, EW], BF16, isOutput=False)
    bq = nc.declare_dram_parameter("bq", [P, 4], F32, isOutput=False)
    bkm = nc.declare_dram_parameter("bkm", [P, 8], F32, isOutput=False)
    sclm = nc.declare_dram_parameter("sclm", [P, 2], F32, isOutput=False)
    bv_bc = nc.declare_dram_parameter("bv_bc", [P, EW], F32, isOutput=False)
    wproj = nc.declare_dram_parameter("wproj", [EW, C], BF16, isOutput=False)
    bproj_bc = nc.declare_dram_parameter("bproj_bc", [P, C], F32, isOutput=False)
    lin1 = nc.declare_dram_parameter("lin1", [C, 4 * C], BF16, isOutput=False)
    blin1 = nc.declare_dram_parameter("blin1", [P, NF], F32, isOutput=False)
    lin2 = nc.declare_dram_parameter("lin2", [4 * C, C], BF16, isOutput=False)
    blin2_bc = nc.declare_dram_parameter("blin2_bc", [P, C], F32, isOutput=False)
    ident = nc.declare_dram_parameter("ident", [P, P], BF16, isOutput=False)
    maskd = nc.declare_dram_parameter("maskd", [P, P], BF16, isOutput=False)
    out = nc.declare_dram_parameter("out", [SROWS, C], F32, isOutput=True)

    xb_t = xb.rearrange("(n p) c -> n p c", p=P)
    xs_t = xsl.rearrange("(n p) c -> n p c", p=P)
    out_t = out.rearrange("(n p) c -> n p c", p=P)
    wq_t = wq.rearrange("(n p) e -> n p e", p=P)
    wk_t = wk.rearrange("(n p) e -> n p e", p=P)
    wv_t = wv.rearrange("(n p) e -> n p e", p=P)
    wp_t = wproj.rearrange("(n p) c -> n p c", p=P)
    l1_t = lin1.rearrange("(n p) f -> n p f", p=P)
    l2_t = lin2.rearrange("(n p) c -> n p c", p=P)

    with tile.TileContext(nc) as tc:
        with (
            tc.tile_pool(name="const", bufs=1) as const,
            tc.tile_pool(name="dram", bufs=1, space="DRAM") as dram,
            tc.tile_pool(name="stat", bufs=3) as stat,
            tc.tile_pool(name="ff0", bufs=1) as ff0,
            tc.tile_pool(name="wA", bufs=4) as wA,
            tc.tile_pool(name="wAf", bufs=2) as wAf,
            tc.tile_pool(name="wB", bufs=2) as wB,
        ):
            # ---- constants (only id_sb DMA'd up front; the rest are
            # deferred behind the first x tiles so LN1 starts asap) ----
            id_sb = const.tile([P, P], BF16)
            nc.sync.dma_start(id_sb[:], ident[:])
            maskd_sb = const.tile([P, P], BF16)
            bq_sb = const.tile([P, 4], F32)
            bkm_sb = const.tile([P, 8], F32)
            scl_sb = const.tile([P, 2], F32)
            bv_sb = const.tile([P, EW], F32)
            bproj_sb = const.tile([P, C], F32)
            blin1_sb = const.tile([P, NF], F32)
            blin2_sb = const.tile([P, C], F32)

            def late_const_dmas():
                nc.sync.dma_start(bq_sb[:], bq[:])
                nc.sync.dma_start(bkm_sb[:], bkm[:])
                nc.sync.dma_start(scl_sb[:], sclm[:])
                nc.sync.dma_start(maskd_sb[:], maskd[:])
                nc.sync.dma_start(bv_sb[:], bv_bc[:])
                nc.sync.dma_start(bproj_sb[:], bproj_bc[:])
                nc.sync.dma_start(blin1_sb[:], blin1[:])
                nc.sync.dma_start(blin2_sb[:], blin2_bc[:])

            ones_f = const.tile([1, 64], F32)
            nc.vector.memset(ones_f[:], 1.0)
            ones_r = const.tile([1, 64], F32R)
            with nc.allow_low_precision(reason="f32r ones for bcast matmul"):
                nc.vector.reciprocal(ones_r[:], ones_f[:])
            eps_sb = const.tile([P, 1], F32)
            nc.vector.memset(eps_sb[:], EPS)
            # preload the sqrt act table so LN1's first sqrt doesn't pay a
            # mid-chain table switch
            tdum = stat.tile([1, 1], F32, tag="tdum")
            nc.scalar.activation(tdum[:], eps_sb[0:1, 0:1], AF.Sqrt)

            rs_in = dram.tile([CUT, C], BF16)
            rs_out = dram.tile([SROWS, C], BF16)
            rs_in_t = rs_in.rearrange("(m p) u -> m p u", p=P)
            rs_in_h = rs_in.rearrange("(h p) u -> h p u", p=SROWS)
            rs_out_t = rs_out.rearrange("(m p) u -> m p u", p=P)
            rs_out_h = rs_out.rearrange("(h p) u -> h p u", p=256)

            res = ff0.tile([P, 4 * C], F32)
            h2T = ff0.tile([P, 4 * C], BF16)   # t-major: col = m*C + j*128 + t
            h2T_r = h2T.rearrange("p (m c) -> p m c", m=4)

            def ln2_stats(k, lnexp, xp, xtag):
                # xf/y2 tiles come from a pool whose slots were recently
                # used by late-phase work: the inherited write-after-read
                # deps stop the scheduler from hoisting this chain (which
                # waits on the collective) into the middle of earlier
                # engine streams (head-of-line blocking).
                ms = (2 * k, 2 * k + 1)
                for m in ms:
                    xt = xp.tile([P, C], F32, tag=xtag)
                    nc.sync.dma_start(xt[:], xs_t[m])
                    nc.vector.tensor_add(
                        res[:, m * C:(m + 1) * C], xt[:], bproj_sb[:]
                    )
                for m in ms:
                    y2 = xp.tile([P, C], BF16, tag=xtag)
                    nc.sync.dma_start(y2[:], rs_out_t[m])
                    rm = res[:, m * C:(m + 1) * C]
                    nc.vector.tensor_add(rm, rm, y2[:])
                return _ln_stats(
                    nc, stat, [res[:, m * C:(m + 1) * C] for m in ms], C,
                    eps_sb[:], lnexp,
                )

            def ln2_finish(k, rstd, nmean, tpool, ttag):
                for i, m in enumerate((2 * k, 2 * k + 1)):
                    rm = res[:, m * C:(m + 1) * C]
                    z2 = wB.tile([P, C], BF16, tag="zt")
                    nc.vector.tensor_scalar(
                        out=z2[:], in0=rm,
                        scalar1=rstd[:, i:i + 1], scalar2=nmean[:, i:i + 1],
                        op0=ALU.mult, op1=ALU.add,
                    )
                    for half in range(2):
                        tp = tpool.tile([P, 512], BF16, tag=ttag)
                        for jj in range(4):
                            j = half * 4 + jj
                            nc.tensor.transpose(
                                tp[:, jj * P:(jj + 1) * P],
                                z2[:, j * P:(j + 1) * P], id_sb[:],
                            )
                        nc.vector.tensor_copy(
                            h2T_r[:, m, half * 512:(half + 1) * 512], tp[:]
                        )
                    nc.vector.tensor_add(rm, rm, blin2_sb[:])

            with tc.tile_pool(name="attA", bufs=1) as attA:
                # kTz: per (hp, r) a [128, T] block at col (2hp+r)*T;
                # r=0 data in rows 0:64, r=1 in rows 64:128, rest zeros.
                kTz = attA.tile([P, 8 * T], BF16)
                qT = attA.tile([P, 4 * CUT], BF16)
                vaug = attA.tile([P, NT * 520], BF16)
                oT = attA.tile([P, 4 * CUT], BF16)

                # ============ LN1 + QKV (pipelined per tile) ============
                with (
                    tc.tile_pool(name="hTp", bufs=2) as hTp,
                    tc.tile_pool(name="wqkv", bufs=1) as wqkv,
                    tc.tile_pool(name="pM", bufs=4, space="PSUM") as pM,
                    tc.tile_pool(name="pT", bufs=2, space="PSUM") as pT,
                ):
                    wq_sb = wqkv.tile([P, NCt * EW], BF16)
                    wk_sb = wqkv.tile([P, NCt * EW], BF16)
                    wv_sb = wqkv.tile([P, NCt * EW], BF16)
                    # weights ride the gpsimd DMA queue (parallel with x);
                    # the big vaug memset comes after so it doesn't delay
                    # the weight issue stream
                    for j in range(NCt):
                        nc.gpsimd.dma_start(
                            wv_sb[:, j * EW:(j + 1) * EW], wv_t[j])
                    for j in range(NCt):
                        nc.gpsimd.dma_start(
                            wk_sb[:, j * EW:(j + 1) * EW], wk_t[j])
                    for j in range(NCt):
                        nc.gpsimd.dma_start(
                            wq_sb[:, j * EW:(j + 1) * EW], wq_t[j])
                    nc.gpsimd.memset(vaug[:], 1.0)

                    grp_tiles = [None] * 4

                    def v_proj(i):
                        g, i4 = divmod(i, 4)
                        gt = grp_tiles[g].rearrange("p (i c) -> p i c", i=4)
                        ps = pM.tile([P, EW], F32, tag="mm")
                        for j in range(NCt):
                            nc.tensor.matmul(
                                ps[:], gt[:, i4, j * P:(j + 1) * P],
                                wv_sb[:, j * EW:(j + 1) * EW],
                                start=(j == 0), stop=(j == NCt - 1),
                            )
                        va = vaug[:, i * 520:(i + 1) * 520].rearrange(
                            "p (h e) -> p h e", e=65
                        )
                        nc.vector.tensor_add(
                            va[:, :, 0:64],
                            ps[:].rearrange("p (h e) -> p h e", e=64),
                            bv_sb[:].rearrange("p (h e) -> p h e", e=64),
                        )

                    # PE warm-up: ~40 full-width transposes on the identity
                    # (>one full 3.4us HAM window) release the clock gate
                    # while the first x tiles and LN1 stats are in flight.
                    for w in range(10):
                        tpw = pT.tile([P, 512], BF16, tag="tp")
                        for jj in range(4):
                            nc.tensor.transpose(
                                tpw[:, jj * P:(jj + 1) * P], id_sb[:],
                                id_sb[:],
                            )

                    for grp in range(4):
                        xts = []
                        for i4 in range(4):
                            xt = wA.tile([P, C], BF16, tag="xt")
                            nc.sync.dma_start(xt[:], xb_t[grp * 4 + i4])
                            xts.append(xt)
                        if grp == 0:
                            late_const_dmas()
                        if grp > 0:
                            rstd, nmean = _ln_stats(
                                nc, stat, [t[:] for t in xts], C, eps_sb[:],
                                lnexp=False,
                            )
                        gtile = hTp.tile([P, 4 * C], BF16, tag="ht")
                        grp_tiles[grp] = gtile
                        gt_r = gtile.rearrange("p (i c) -> p i c", i=4)
                        for i4 in range(4):
                            i = grp * 4 + i4
                            if grp == 0:
                                # per-tile stats so tile 0 unblocks asap
                                rstd, nmean = _ln_stats(
                                    nc, stat, [xts[i4][:]], C, eps_sb[:],
                                    lnexp=False,
                                )
                                b_ap, s_ap = nmean[:, 0:1], rstd[:, 0:1]
                            else:
                                b_ap = nmean[:, i4:i4 + 1]
                                s_ap = rstd[:, i4:i4 + 1]
                            zt = wB.tile([P, C], BF16, tag="zt")
                            nc.vector.tensor_scalar(
                                out=zt[:], in0=xts[i4][:],
                                scalar1=s_ap, scalar2=b_ap,
                                op0=ALU.mult, op1=ALU.add,
                            )
                            for half in range(2):
                                tp = pT.tile([P, 512], BF16, tag="tp")
                                for jj in range(4):
                                    j = half * 4 + jj
                                    nc.tensor.transpose(
                                        tp[:, jj * P:(jj + 1) * P],
                                        zt[:, j * P:(j + 1) * P], id_sb[:],
                                    )
                                nc.vector.tensor_copy(
                                    gt_r[:, i4, half * 512:(half + 1) * 512],
                                    tp[:],
                                )
                            if i > 0:
                                v_proj(i - 1)
                        # K projection for this t-chunk -> kTz halves
                        for hp in range(4):
                            ps = pM.tile([P, 512], F32, tag="mm")
                            for j in range(NCt):
                                nc.tensor.matmul(
                                    ps[:],
                                    wk_sb[:, j * EW + hp * P:
                                          j * EW + (hp + 1) * P],
                                    gt_r[:, 0:4, j * P:(j + 1) * P],
                                    start=(j == 0), stop=(j == NCt - 1),
                                )
                            # r=0 half on Act, r=1 on Vector (load balance)
                            blk0 = (2 * hp) * T
                            nc.scalar.activation(
                                kTz[:, blk0 + grp * 512:
                                    blk0 + (grp + 1) * 512],
                                ps[:], AF.Identity,
                                bias=bkm_sb[:, 2 * hp:2 * hp + 1],
                                scale=scl_sb[:, 0:1],
                            )
                            blk1 = (2 * hp + 1) * T
                            nc.vector.tensor_scalar(
                                out=kTz[:, blk1 + grp * 512:
                                        blk1 + (grp + 1) * 512],
                                in0=ps[:],
                                scalar1=scl_sb[:, 1:2],
                                scalar2=bkm_sb[:, 2 * hp + 1:2 * hp + 2],
                                op0=ALU.mult, op1=ALU.add,
                            )
                        if grp >= 2:
                            sc = grp - 2
                            for hp in range(4):
                                ps = pM.tile([P, 512], F32, tag="mm")
                                for j in range(NCt):
                                    nc.tensor.matmul(
                                        ps[:],
                                        wq_sb[:, j * EW + hp * P:
                                              j * EW + (hp + 1) * P],
                                        gt_r[:, 0:4, j * P:(j + 1) * P],
                                        start=(j == 0), stop=(j == NCt - 1),
                                    )
                                nc.scalar.activation(
                                    qT[:, hp * CUT + sc * 512:
                                       hp * CUT + (sc + 1) * 512],
                                    ps[:], AF.Identity,
                                    bias=bq_sb[:, hp:hp + 1],
                                )
                    v_proj(NT - 1)

                # ============ attention ============
                with (
                    tc.tile_pool(name="wpj", bufs=1) as wpj,
                    tc.tile_pool(name="wC", bufs=3) as wC,
                    tc.tile_pool(name="pS", bufs=2, space="PSUM") as pS,
                    tc.tile_pool(name="pO", bufs=2, space="PSUM") as pO,
                    tc.tile_pool(name="pX", bufs=2, space="PSUM") as pX,
                ):
                    wp_sb = wpj.tile([P, 4 * C], BF16)
                    for et in range(4):
                        nc.gpsimd.dma_start(
                            wp_sb[:, et * C:(et + 1) * C], wp_t[et])

                    def attn_group(sc, hp, r):
                        n_vis = 12 + 4 * sc
                        n_full = n_vis - 4
                        hh = 2 * hp + r
                        kblk = hh * T
                        ops = pO.tile([65, 512], F32, tag="ops")

                        def qsl(a, b):
                            return qT[:, hp * CUT + sc * 512 + a:
                                      hp * CUT + sc * 512 + b]

                        for p2 in range(n_full // 2):
                            sps = pS.tile([P, 1024], F32, tag="sm")
                            for half in range(2):
                                tt = 2 * p2 + half
                                nc.tensor.matmul(
                                    sps[:, half * 512:(half + 1) * 512],
                                    kTz[:, kblk + tt * P:kblk + (tt + 1) * P],
                                    qsl(0, 512), start=True, stop=True,
                                )
                            pt = wC.tile([P, 1024], BF16, tag="pt")
                            nc.scalar.activation(
                                pt[:], sps[:], AF.Exp, scale=ATT_SCALE)
                            for half in range(2):
                                tt = 2 * p2 + half
                                nc.tensor.matmul(
                                    ops[:],
                                    vaug[:, tt * 520 + hh * 65:
                                         tt * 520 + (hh + 1) * 65],
                                    pt[:, half * 512:(half + 1) * 512],
                                    start=(tt == 0), stop=False,
                                    skip_group_check=True,
                                )
                        # boundary tiles: tight-packed into 2 psum tiles so
                        # each pair needs ONE exp instruction
                        for bp in range(2):
                            kbs = (2 * bp, 2 * bp + 1)
                            los = [kb * P for kb in kbs]
                            ws = [512 - lo for lo in los]
                            c0s = [0, ws[0]]
                            W = ws[0] + ws[1]
                            sps = pS.tile([P, 1024], F32, tag="sm")
                            for x2, kb in enumerate(kbs):
                                tt = n_full + kb
                                nc.tensor.matmul(
                                    sps[:, c0s[x2]:c0s[x2] + ws[x2]],
                                    kTz[:, kblk + tt * P:
                                        kblk + (tt + 1) * P],
                                    qsl(los[x2], 512), start=True, stop=True,
                                )
                            pt = wC.tile([P, 1024], BF16, tag="pt")
                            nc.scalar.activation(
                                pt[:, 0:W], sps[:, 0:W], AF.Exp,
                                scale=ATT_SCALE,
                            )
                            for x2, kb in enumerate(kbs):
                                tt = n_full + kb
                                nc.vector.tensor_mul(
                                    pt[:, c0s[x2]:c0s[x2] + P],
                                    pt[:, c0s[x2]:c0s[x2] + P],
                                    maskd_sb[:],
                                )
                                nc.tensor.matmul(
                                    ops[:, los[x2]:512],
                                    vaug[:, tt * 520 + hh * 65:
                                         tt * 520 + (hh + 1) * 65],
                                    pt[:, c0s[x2]:c0s[x2] + ws[x2]],
                                    start=False,
                                    stop=(kb == 3),
                                    skip_group_check=True,
                                )
                        # normalize: denom bcast + fast reciprocal
                        denb = stat.tile([1, 512], F32R, tag="denb")
                        with nc.allow_low_precision(reason="f32r denom copy"):
                            nc.scalar.activation(
                                denb[:], ops[64:65, :], AF.Identity)
                        rb = pX.tile([P, 512], F32, tag="aux")
                        nc.tensor.matmul(
                            rb[0:64, :], ones_r[:], denb[:],
                            start=True, stop=True,
                        )
                        rcp = wB.tile([64, 512], F32, tag="rcp")
                        nc.vector.reciprocal_approx_fast(rcp[:], rb[0:64, :])
                        nc.vector.tensor_mul(
                            oT[64 * r:64 * (r + 1),
                               hp * CUT + sc * 512:hp * CUT + (sc + 1) * 512],
                            ops[0:64, :], rcp[:],
                        )

                    def proj_m(m):
                        yst = wB.tile([P, C], BF16, tag="yst")
                        for nh in range(2):
                            psx = pX.tile([P, 512], F32, tag="aux")
                            for et in range(4):
                                nc.tensor.matmul(
                                    psx[:],
                                    oT[:, et * CUT + m * P:
                                       et * CUT + (m + 1) * P],
                                    wp_sb[:, et * C + nh * 512:
                                          et * C + (nh + 1) * 512],
                                    start=(et == 0), stop=(et == 3),
                                )
                            nc.vector.tensor_copy(
                                yst[:, nh * 512:(nh + 1) * 512], psx[:])
                        nc.sync.dma_start(rs_in_t[m], yst[:])

                    for sc in range(2):
                        for hp in range(4):
                            for r in range(2):
                                attn_group(sc, hp, r)
                        if sc == 1:
                            # LN2 chunk 0: RS0 completed mid-attention; the
                            # wC/"pt" slots carry late-sc1 deps so the chain
                            # is scheduled at attention end, overlapping the
                            # proj matmuls below.
                            ln2s0 = ln2_stats(0, False, wC, "pt")
                        for m in range(4 * sc, 4 * sc + 4):
                            proj_m(m)
                        if sc == 1:
                            ln2_finish(0, ln2s0[0], ln2s0[1], pS, "sm")
                        nc.gpsimd.collective_compute(
                            "ReduceScatter",
                            ALU.add,
                            replica_groups=[[0, 1], [2, 3], [4, 5], [6, 7]],
                            ins=[rs_in_h[sc]],
                            outs=[rs_out_h[sc]],
                        )

            # ============ FFN ============
            with (
                tc.tile_pool(name="ffG", bufs=1) as ffG,
                tc.tile_pool(name="l1p", bufs=2) as l1p,
                tc.tile_pool(name="ffW", bufs=3) as ffW,
            ):
                gT = ffG.tile([P, NF * SROWS], BF16)
                QF = 8  # f-tiles per streamed lin1 quarter

                with (
                    tc.tile_pool(name="pG", bufs=3, space="PSUM") as pG,
                    tc.tile_pool(name="pT2", bufs=1, space="PSUM") as pT2,
                    tc.tile_pool(name="pF", bufs=1, space="PSUM") as pF,
                ):
                    def ffn1_chunk(k):
                        for ft in range(NF):
                            fl = ft % QF
                            if fl == 0:
                                l1q = l1p.tile(
                                    [P, NCt * QF * P], BF16, tag="l1")
                                q0 = (ft // QF) * QF * P
                                for j in range(NCt):
                                    nc.gpsimd.dma_start(
                                        l1q[:, j * QF * P:(j + 1) * QF * P],
                                        l1_t[j][:, q0:q0 + QF * P],
                                    )
                            ps = pG.tile([P, 256], F32, tag="g")
                            for j in range(NCt):
                                nc.tensor.matmul(
                                    ps[:],
                                    l1q[:, j * QF * P + fl * P:
                                         j * QF * P + (fl + 1) * P],
                                    h2T_r[:, 2 * k:2 * k + 2,
                                          j * P:(j + 1) * P],
                                    start=(j == 0), stop=(j == NCt - 1),
                                )
                            nc.scalar.activation(
                                gT[:, ft * 512 + k * 256:
                                   ft * 512 + (k + 1) * 256],
                                ps[:], AF.Gelu, bias=blin1_sb[:, ft:ft + 1],
                            )

                    def ffn2_pass(ms2):
                        # lin2 streamed per pass; 4 psum accumulators
                        fps = {}
                        for m in ms2:
                            for nh in range(2):
                                fpt = pF.tile([P, 512], F32,
                                              tag=f"f{2 * (m % 2) + nh}")
                                fps[2 * m + nh] = fpt
                        for ft in range(NF):
                            l2f = ffW.tile([P, C], BF16, tag="l2")
                            nc.scalar.dma_start(l2f[:], l2_t[ft])
                            for m in ms2:
                                ck, mw = divmod(m, 2)
                                gsl = gT[:, ft * 512 + ck * 256 + mw * P:
                                         ft * 512 + ck * 256 + (mw + 1) * P]
                                for nh in range(2):
                                    nc.tensor.matmul(
                                        fps[2 * m + nh][:],
                                        gsl,
                                        l2f[:, nh * 512:(nh + 1) * 512],
                                        start=(ft == 0), stop=(ft == NF - 1),
                                    )
                                if ft == NF - 1:
                                    # drain each accumulator immediately
                                    for nh in range(2):
                                        o_sb = ffW.tile(
                                            [P, 512], F32, tag="osb")
                                        nc.vector.tensor_add(
                                            o_sb[:], fps[2 * m + nh][:],
                                            res[:, m * C + nh * 512:
                                                m * C + (nh + 1) * 512],
                                        )
                                        nc.sync.dma_start(
                                            out_t[m][:, nh * 512:
                                                     (nh + 1) * 512],
                                            o_sb[:])

                    # FFN2 over chunk-0 rows runs right after FFN1 chunk 0;
                    # the RS1-dependent LN2 chunk 1 (slotted late via the
                    # l1p quarter-tile slot deps) hides under it.
                    ffn1_chunk(0)
                    ffn2_pass([0, 1])
                    st1 = ln2_stats(1, False, l1p, "l1")
                    ln2_finish(1, st1[0], st1[1], pT2, "tp2")
                    ffn1_chunk(1)
                    ffn2_pass([2, 3])

    nc.compile()
    return nc


_NC = None


def _get_nc():
    global _NC
    if _NC is None:
        _NC = build_nc()
    return _NC


def kernel(**inputs):
    nc = _get_nc()
    bf = ml_dtypes.bfloat16
    f32 = np.float32

    x = np.asarray(inputs["x"], f32)
    Wq = np.asarray(inputs["Wq"], f32)
    Wk = np.asarray(inputs["Wk"], f32)
    Wv = np.asarray(inputs["Wv"], f32)
    bq = np.asarray(inputs["bq"], f32)
    bk = np.asarray(inputs["bk"], f32)
    bv = np.asarray(inputs["bv"], f32)
    proj_w = np.asarray(inputs["proj_w"], f32)
    proj_b = np.asarray(inputs["proj_b"], f32)
    ln1_w = np.asarray(inputs["ln1_w"], f32)
    ln1_b = np.asarray(inputs["ln1_b"], f32)
    ln2_w = np.asarray(inputs["ln2_w"], f32)
    ln2_b = np.asarray(inputs["ln2_b"], f32)
    lin1_w = np.asarray(inputs["lin1_w"], f32)
    lin1_b = np.asarray(inputs["lin1_b"], f32)
    lin2_w = np.asarray(inputs["lin2_w"], f32)
    lin2_b = np.asarray(inputs["lin2_b"], f32)

    ident = np.eye(P, dtype=bf)
    kl = np.arange(P)[:, None]
    ql = np.arange(P)[None, :]
    maskd = (ql >= kl).astype(bf)

    # fold ln1 affine into QKV, ln2 affine into lin1
    Wq_f = Wq * ln1_w[None, :, None]
    Wk_f = Wk * ln1_w[None, :, None]
    Wv_f = Wv * ln1_w[None, :, None]
    bq_f = bq + np.einsum("c,hcd->hd", ln1_b, Wq)
    bk_f = bk + np.einsum("c,hcd->hd", ln1_b, Wk)
    bv_f = bv + np.einsum("c,hcd->hd", ln1_b, Wv)
    lin1_f = (lin1_w * ln2_w[:, None]).astype(bf)
    blin1_f = lin1_b + ln2_b @ lin1_w
    blin1_t = np.ascontiguousarray(blin1_f.reshape(NF, P).T).astype(f32)
    blin2_bc = np.ascontiguousarray(np.broadcast_to(lin2_b, (P, C))).astype(f32)
    lin2_bf = lin2_w.astype(bf)
    proj_w_bf = proj_w.astype(bf)
    bproj_bc = np.ascontiguousarray(np.broadcast_to(proj_b, (P, C))).astype(f32)

    # 0/1 scale masks for writing kTz halves (col r: rows 64r..64r+63 = 1)
    sclm = np.zeros((P, 2), f32)
    sclm[0:64, 0] = 1.0
    sclm[64:128, 1] = 1.0

    x_bf = [np.ascontiguousarray(x[b]).astype(bf) for b in range(B)]

    in_maps = []
    for c in range(8):
        b, g = divmod(c, 2)
        hsl = slice(g * GH, (g + 1) * GH)
        wq_c = np.ascontiguousarray(
            Wq_f[hsl].transpose(1, 0, 2).reshape(C, EW)).astype(bf)
        wk_c = np.ascontiguousarray(
            Wk_f[hsl].transpose(1, 0, 2).reshape(C, EW)).astype(bf)
        wv_c = np.ascontiguousarray(
            Wv_f[hsl].transpose(1, 0, 2).reshape(C, EW)).astype(bf)
        bq_c = np.ascontiguousarray(bq_f[hsl].reshape(4, P).T).astype(f32)
        # bkm: col 2hp+r holds head (hp,r)'s bias in its own 64-row half,
        # zeros in the other half (matches the kTz write masks)
        bk_pair = bk_f[hsl].reshape(4, P).T  # [128, 4], col hp
        bkm = np.zeros((P, 8), f32)
        for hp in range(4):
            bkm[0:64, 2 * hp] = bk_pair[0:64, hp]
            bkm[64:128, 2 * hp + 1] = bk_pair[64:128, hp]
        bv_c = np.ascontiguousarray(
            np.broadcast_to(bv_f[hsl].reshape(EW), (P, EW))).astype(f32)
        wproj_c = np.ascontiguousarray(proj_w_bf[g * EW:(g + 1) * EW, :])
        r0 = T - CUT + g * 256
        r1 = T - CUT + 512 + g * 256
        xs = np.concatenate([x[b, r0:r0 + 256], x[b, r1:r1 + 256]], axis=0)
        in_maps.append({
            "xb": x_bf[b],
            "xslice": np.ascontiguousarray(xs),
            "wq": wq_c, "wk": wk_c, "wv": wv_c,
            "bq": bq_c, "bkm": bkm, "sclm": sclm, "bv_bc": bv_c,
            "wproj": wproj_c, "bproj_bc": bproj_bc,
            "lin1": lin1_f, "blin1": blin1_t,
            "lin2": lin2_bf, "blin2_bc": blin2_bc,
            "ident": ident, "maskd": maskd,
        })

    r = run_bass_kernel_spmd(nc, in_maps, core_ids=list(range(8)))
    out_full = np.empty((B, CUT, C), f32)
    for c in range(8):
        b, g = divmod(c, 2)
        o = r.results[c]["out"]
        out_full[b, g * 256:(g + 1) * 256, :] = o[0:256]
        out_full[b, 512 + g * 256:512 + (g + 1) * 256, :] = o[256:512]
    return out_full
